# revision 16
# baseline (speedup 1.0000x reference)
"""AttentionPooling Bass kernel for 8 TRN2 NeuronCores.

Problem: x [262144, 1024] f32, bags of 128 consecutive rows (2048 bags).
  scores = (tanh(x @ W1 + b1) @ W2 + b2)[:, 0]        per-row MLP score
  w      = softmax(scores) within each bag
  out[b] = sum_i w[i] * x[i]  over the bag's rows  -> [2048, 1024] f32

Sharding: data-parallel over bags; core c gets bags [c*256, (c+1)*256).
Weights replicated. No cross-core communication. b2 is dropped (uniform
shift inside each bag's softmax — a no-op for the output).

Numerics: |score| <= ||W2||_1 < 26 (tanh-bounded), so exp() cannot
overflow f32 and the softmax max-subtraction is skipped entirely.
Normalization is deferred to the HOST: the device returns per-bag
unnormalized weighted sums plus the per-bag sum of exp (den); kernel()
divides on the way out.

Mixed precision phase 1 (the ~89%-of-PE-time term): the last M_FP8
bags of each 8-bag group run their score matmul fully in fp8-e4m3
with DoubleRow (K=256 per matmul at 2 elem/cycle — 2x bf16), the rest
in bf16. Bag-granular (not chunk-granular) keeps the PE dtype
transitions to ~2 per group — interleaving dtypes per-bag measurably
costs ~200ns per transition. Operands are pre-scaled (xt*16, W1*512 —
exact pow2 shifts) so both dtypes produce PSUM = 8192*h and tanh
applies scale=1/8192. fp8 on a fraction f of the bags costs output
rel err ~ 0.026*sqrt(f) (numpy-sim, HW-confirmed) vs the 2e-2 gate:
M_FP8=4 measures 0.0181.

Pipeline notes (learned from traces):
- out-row/den DMAs live on the gpsimd queue: on the sync queue they
  head-of-line blocked the next group's input DMAs at every group
  boundary (the sync queue runs ~10us ahead, in order).
- gpsimd.partition_broadcast loads a GPSIMD ucode library whose
  LOAD_LIB + 17us DRAIN stalls the gpsimd DMA queue at startup; w2 is
  replicated with a stride-0 broadcast DMA read instead.
- the softmax chain is split per QUARTET (4 bags): exp of bags
  [4q,4q+4) only needs those bags' score reductions, so quartet 0 of
  group g runs at bag 6 of group g and quartet 1 at bag 2 of group
  g+1 — no cross-engine chain dangles off the group boundary.
"""

import sys

if "/opt/trn_rl_repo" not in sys.path:
    sys.path.insert(0, "/opt/trn_rl_repo")

import numpy as np

import concourse.bass as bass
import concourse.bacc as bacc
import concourse.mybir as mybir
import concourse.tile as tile
from concourse.bass_utils import run_bass_kernel_spmd

F32 = mybir.dt.float32
BF16 = mybir.dt.bfloat16
FP8 = mybir.dt.float8e4
AF = mybir.ActivationFunctionType
ALU = mybir.AluOpType
DR = mybir.MatmulPerfMode.DoubleRow

N_CORES = 8
BAG = 128
D = 1024
H = 1024
DC = D // 128   # contraction chunks
GROUP = 8       # bags per group
WG = 4          # bags per weighted-sum quartet (PSUM col-group packing)

# bags per group (of 8) whose score matmul runs fully in fp8-e4m3
# DoubleRow (the LAST M_FP8 bags of each group — consecutive to
# minimize PE dtype switches, last so the first bags only need the
# bf16 weights that stream first). 0 = pure bf16.
# Output rel err ~ sqrt(M_FP8/8)*0.026.
M_FP8 = 4

SX = 16.0       # x pre-scale (exact pow2)
SW = 512.0      # W1 pre-scale (exact pow2)

# set by test.py for profiling; the grading harness leaves these alone
TRACE = False
LAST_EXEC_NS = None
LAST_PROFILE = None

_cache = {}


def _build(bags_core: int, with_b1: bool, m_fp8: int, n_cores: int = N_CORES):
    """Build the per-core Bass module. All cores run the same NEFF."""
    assert bags_core % GROUP == 0 and GROUP % WG == 0
    assert 0 <= m_fp8 <= GROUP
    rows_core = bags_core * BAG
    n_groups = bags_core // GROUP
    use_fp8 = m_fp8 > 0
    use_bf = m_fp8 < GROUP

    nc = bacc.Bacc("TRN2", target_bir_lowering=False, debug=False,
                   num_devices=n_cores)
    x_h = nc.declare_dram_parameter("x", [rows_core, D], BF16, isOutput=False)
    if use_bf:
        xtb_h = nc.declare_dram_parameter("xtb", [rows_core, D], BF16,
                                          isOutput=False)
        w1b_h = nc.declare_dram_parameter("w1b", [D, H], BF16, isOutput=False)
    if use_fp8:
        xtf_h = nc.declare_dram_parameter("xtf", [rows_core, D], FP8,
                                          isOutput=False)
        w1f_h = nc.declare_dram_parameter("w1f", [D, H], FP8, isOutput=False)
    w2_h = nc.declare_dram_parameter("w2", [1, H], BF16, isOutput=False)
    if with_b1:
        b1_h = nc.declare_dram_parameter("b1", [1, H], BF16, isOutput=False)
    out_h = nc.declare_dram_parameter("out", [bags_core, D], F32, isOutput=True)
    den_h = nc.declare_dram_parameter("den", [bags_core, 1], F32, isOutput=True)

    with tile.TileContext(nc) as tc:
        with (
            tc.tile_pool(name="const", bufs=1) as const_pool,
            tc.tile_pool(name="xtb", bufs=6) as xtb_pool,
            tc.tile_pool(name="xtf", bufs=6) as xtf_pool,
            tc.tile_pool(name="xb", bufs=16) as xb_pool,
            tc.tile_pool(name="tanh", bufs=2) as t_pool,
            tc.tile_pool(name="dump", bufs=1) as dump_pool,
            tc.tile_pool(name="scores", bufs=2) as sc_pool,
            tc.tile_pool(name="e4", bufs=3) as e_pool,
            tc.tile_pool(name="den", bufs=3) as den_pool,
            tc.tile_pool(name="ystage", bufs=2) as y_pool,
            tc.tile_pool(name="ps_s", bufs=2, space="PSUM") as ps_s_pool,
            tc.tile_pool(name="ps_y", bufs=3, space="PSUM") as ps_y_pool,
            tc.tile_pool(name="ps_d", bufs=1, space="PSUM") as ps_d_pool,
        ):
            # ---- constants / weights (resident) ----
            ones_col = const_pool.tile([128, 1], BF16)
            nc.any.memset(ones_col[:, :], 1.0)

            # HAM warmup: ~3.5us of dummy matmuls (M=1, garbage rhs,
            # output never read) issued while the weight/input DMAs
            # stream, so the real matmul stream starts at the warm
            # 2.4GHz clock instead of paying the cold ramp.
            warm_rhs = const_pool.tile([128, 512], BF16)
            nc.any.memset(warm_rhs[:, :], 0.0)
            ps_warm = ps_y_pool.tile([128, 512], F32, name="ps_y")
            for _ in range(16):
                nc.tensor.matmul(ps_warm[0:1, :], lhsT=ones_col[:, :],
                                 rhs=warm_rhs[:, :], start=True, stop=True)

            # Weight DMA order matches first use: bag 0 (bf16) sweeps
            # w1b chunks 0..7 of the j=0 half first, so stream those
            # halves before anything else; w1f is first needed by bag
            # 8-M_FP8 (~4 bag-periods in).
            if use_bf:
                w1b_sb = const_pool.tile([128, DC, H], BF16)
                for j in range(2):
                    for c in range(DC):
                        nc.gpsimd.dma_start(
                            out=w1b_sb[:, c, 512 * j:512 * (j + 1)],
                            in_=w1b_h[c * 128:(c + 1) * 128,
                                      512 * j:512 * (j + 1)])
            if use_fp8:
                w1f_sb = const_pool.tile([128, DC, H], FP8)
                for c in range(DC):
                    nc.gpsimd.dma_start(out=w1f_sb[:, c, :],
                                        in_=w1f_h[c * 128:(c + 1) * 128, :])

            # replicate W2 across partitions with a stride-0 DMA read
            w2_rep = const_pool.tile([128, H], BF16)
            nc.gpsimd.dma_start(out=w2_rep[:, :],
                                in_=w2_h[:, :].broadcast_to([128, H]))

            if with_b1:
                b1_row = const_pool.tile([1, H], BF16)
                nc.gpsimd.dma_start(out=b1_row[:, :], in_=b1_h[:, :])
                ones_row = const_pool.tile([1, 128], BF16)
                nc.any.memset(ones_row[:, :], 1.0)

            def phase1(g, sc_tile, xbs, mid_cbs):
                """Score matmuls+tanh+projection for group g's 8 bags.

                Fills sc_tile [128, 8] (per-row scores, column=bag) and
                appends the bags' x tiles to xbs. mid_cbs: {bag_idx:
                [callbacks]} invoked between bags to stage quartet
                work inside this group's matmul stream.
                """
                for n in range(GROUP):
                    for cb in mid_cbs.get(n, ()):
                        cb()
                    bag = g * GROUP + n
                    is8 = n >= GROUP - m_fp8
                    rs = slice(bag * BAG, (bag + 1) * BAG)
                    if is8:
                        xt_t = xtf_pool.tile([128, DC, 128], FP8)
                        nc.sync.dma_start(out=xt_t[:, :, :], in_=xtf_h[rs, :])
                    else:
                        xt_t = xtb_pool.tile([128, DC, 128], BF16)
                        nc.sync.dma_start(out=xt_t[:, :, :], in_=xtb_h[rs, :])
                    x_b = xb_pool.tile([128, D], BF16)
                    nc.sync.dma_start(out=x_b[:, :], in_=x_h[rs, :])
                    xbs.append(x_b)

                    ps_s = ps_s_pool.tile([128, 2, 512], F32)
                    for j in range(2):
                        ps_j = ps_s[:, j, :]
                        hs = slice(512 * j, 512 * (j + 1))
                        if is8:
                            for p in range(DC // 2):
                                nc.tensor.matmul(ps_j[:, :],
                                                 lhsT=xt_t[:, 2 * p:2 * p + 2, :],
                                                 rhs=w1f_sb[:, 2 * p:2 * p + 2, hs],
                                                 start=(p == 0),
                                                 stop=(p == DC // 2 - 1
                                                       and not with_b1),
                                                 perf_mode=DR)
                        else:
                            for c in range(DC):
                                nc.tensor.matmul(ps_j[:, :],
                                                 lhsT=xt_t[:, c, :],
                                                 rhs=w1b_sb[:, c, hs],
                                                 start=(c == 0),
                                                 stop=(c == DC - 1
                                                       and not with_b1))
                        if with_b1:
                            nc.tensor.matmul(ps_j[:, :], lhsT=ones_row[:, :],
                                             rhs=b1_row[:, hs],
                                             start=False, stop=True)
                    t_t = t_pool.tile([128, H], BF16)
                    # one activation spanning both PSUM banks: ScalarE is
                    # near-saturated during the fp8 half, and per-op
                    # overhead (~150ns) on 2 ops/bag was gating the ps_s
                    # recycle that the next bags' matmuls wait on
                    nc.scalar.activation(t_t[:, :], ps_s[:, :, :], AF.Tanh,
                                         scale=1.0 / (SX * SW))

                    dump = dump_pool.tile([128, H], BF16)
                    nc.vector.tensor_mul(dump[:, :], t_t[:, :], w2_rep[:, :])
                    nc.vector.reduce_sum(sc_tile[:, n:n + 1], dump[:, :],
                                         axis=mybir.AxisListType.X)

            def q_prep(q, sc_tile, e8):
                """exp of bags [4q,4q+4) into e8[:, q, :], plus the 8x
                column-replicated copy the weighted-sum matmuls load:
                col c of the M=32 stationary holds e[:, c mod 4]; bag
                (q,v)'s out row lands at partition 32v + (4q+v) as
                (4q+v) mod 4 == v."""
                nc.scalar.activation(e8[:, q, :],
                                     sc_tile[:, q * WG:(q + 1) * WG], AF.Exp)
                e32_t = e_pool.tile([128, GROUP, WG], BF16)
                nc.vector.tensor_copy(
                    e32_t[:, :, :],
                    e8[:, q, :].unsqueeze(1).broadcast_to([128, GROUP, WG]))
                return e32_t

            def q_wsum(g, q, xbs, e32_t, e8, last=False):
                """weighted sums for bags [4q,4q+4) of group g; quartet 1
                also emits the group's denominator matmul."""
                e32 = e32_t[:, :, :]
                if q == 1:
                    ps_d = ps_d_pool.tile([128, 1], F32)
                    nc.tensor.matmul(ps_d[0:GROUP, :], lhsT=e8[:, :, :],
                                     rhs=ones_col[:, :], start=True, stop=True)
                    dstage = den_pool.tile([GROUP, 1], F32)
                    nc.vector.tensor_copy(dstage[:, :], ps_d[0:GROUP, :])
                    b0 = g * GROUP
                    nc.gpsimd.dma_start(out=den_h[b0:b0 + GROUP, :],
                                        in_=dstage[:, :])

                ys = y_pool.tile([128, D], F32)
                for j in range(2):
                    ps_y = ps_y_pool.tile([128, 512], F32)
                    for v in range(WG):
                        b = q * WG + v
                        nc.tensor.matmul(ps_y[32 * v:32 * v + 32, :],
                                         lhsT=e32,
                                         rhs=xbs[b][:, 512 * j:512 * (j + 1)],
                                         start=True, stop=True,
                                         tile_position=(0, 32 * v))
                    # Alternate ScalarE/VectorE so the two banks drain in
                    # parallel (bank fully written so ScalarE is safe).
                    if j == 0:
                        nc.vector.tensor_copy(ys[:, 0:512], ps_y[:, :])
                    else:
                        nc.scalar.copy(ys[:, 512:1024], ps_y[:, :])
                # spread the final quartet's row DMAs across queues so the
                # kernel tail isn't serialized on one DMA queue
                engines = ([nc.gpsimd, nc.sync, nc.scalar, nc.gpsimd]
                           if last else [nc.gpsimd] * WG)
                for v in range(WG):
                    bag = g * GROUP + q * WG + v
                    p = 32 * v + q * WG + v
                    engines[v].dma_start(out=out_h[bag:bag + 1, :],
                                         in_=ys[p:p + 1, :])

            # Pipeline: quartet 0 of group g: exp+replicate at bag 5
            # (bags 0-3's reductions are just done), weighted sums at
            # bag 7 (the stationary is ready well before its LDW).
            # Quartet 1 of group g: exp+replicate at bag 1 of g+1,
            # weighted sums + denominator at bag 3 of g+1.
            prev = None
            for g in range(n_groups):
                sc_tile = sc_pool.tile([128, GROUP], F32)
                e8 = e_pool.tile([128, 2, WG], BF16, name="e8")
                xbs = []
                cell = {}

                def p0(s=sc_tile, e=e8, c=cell):
                    c["e0"] = q_prep(0, s, e)

                def w0(g=g, x=xbs, e=e8, c=cell):
                    q_wsum(g, 0, x, c["e0"], e)

                cbs = {5: [p0], 7: [w0]}
                if prev is not None:
                    pg, psc, pxbs, pe8, pcell = prev

                    def p1(s=psc, e=pe8, c=pcell):
                        c["e1"] = q_prep(1, s, e)

                    def w1(pg=pg, x=pxbs, e=pe8, c=pcell):
                        q_wsum(pg, 1, x, c["e1"], e)

                    cbs[1] = [p1]
                    cbs[3] = [w1]
                phase1(g, sc_tile, xbs, cbs)
                prev = (g, sc_tile, xbs, e8, cell)
            pg, psc, pxbs, pe8, pcell = prev
            e1 = q_prep(1, psc, pe8)
            q_wsum(pg, 1, pxbs, e1, pe8, last=True)

    nc.finalize()
    return nc


def _numpy_fallback(x, W1, b1, W2, b2, bag_sizes):
    seg_ends = np.cumsum(bag_sizes)
    seg_starts = seg_ends - bag_sizes
    scores = (np.tanh(x @ W1 + b1) @ W2 + b2)[:, 0]
    out = np.zeros((bag_sizes.shape[0], x.shape[1]), dtype=x.dtype)
    for i, (s, e) in enumerate(zip(seg_starts, seg_ends)):
        sc = scores[s:e]
        w = np.exp(sc - sc.max())
        w /= w.sum()
        out[i] = w @ x[s:e]
    return out


def _host_prep(x, n_bags, m_fp8):
    """bf16 cast of x, plus the per-bag-transposed scaled copies
    (bf16 and/or fp8 depending on the bag mix) via jax CPU."""
    import jax
    import jax.numpy as jnp
    import ml_dtypes

    cpu = jax.devices("cpu")[0]
    with jax.default_device(cpu):
        xj = jnp.asarray(x)
        xb = np.asarray(xj.astype(jnp.bfloat16))
        xt = ((xj * SX).reshape(n_bags, BAG, DC, 128).transpose(0, 3, 2, 1)
              .reshape(n_bags * BAG, D))
        xtb = xtf = None
        if m_fp8 < GROUP:
            xtb = np.asarray(xt.astype(jnp.bfloat16))
        if m_fp8 > 0:
            xtf = np.asarray(xt).astype(ml_dtypes.float8_e4m3)
        return xb, xtb, xtf


def kernel(x, W1, b1, W2, b2, bag_sizes):
    x = np.ascontiguousarray(np.asarray(x, dtype=np.float32))
    W1 = np.asarray(W1, dtype=np.float32)
    b1 = np.asarray(b1, dtype=np.float32)
    W2 = np.asarray(W2, dtype=np.float32)
    b2 = np.asarray(b2, dtype=np.float32)
    bag_sizes = np.asarray(bag_sizes)

    n_bags = bag_sizes.shape[0]
    if not (np.all(bag_sizes == BAG) and x.shape[0] == n_bags * BAG
            and x.shape[1] == D and n_bags % (N_CORES * GROUP) == 0):
        return _numpy_fallback(x, W1, b1, W2, b2, bag_sizes)

    bags_core = n_bags // N_CORES
    rows_core = bags_core * BAG
    with_b1 = bool(np.any(b1))

    key = (bags_core, with_b1, M_FP8)
    if key not in _cache:
        _cache[key] = _build(bags_core, with_b1, M_FP8)
    nc = _cache[key]

    import ml_dtypes
    x_bf, xtb, xtf = _host_prep(x, n_bags, M_FP8)
    w1s = W1 * SW
    w2_row = np.ascontiguousarray(W2.reshape(1, H)).astype(ml_dtypes.bfloat16)
    in_maps = []
    for c in range(N_CORES):
        rs = slice(c * rows_core, (c + 1) * rows_core)
        im = {"x": x_bf[rs], "w2": w2_row}
        if M_FP8 < GROUP:
            im["xtb"] = xtb[rs]
            im["w1b"] = w1s.astype(ml_dtypes.bfloat16)
        if M_FP8 > 0:
            im["xtf"] = xtf[rs]
            im["w1f"] = w1s.astype(ml_dtypes.float8_e4m3)
        if with_b1:
            im["b1"] = np.ascontiguousarray(
                (b1 * SX * SW).reshape(1, H)).astype(ml_dtypes.bfloat16)
        in_maps.append(im)

    res = run_bass_kernel_spmd(nc, in_maps, core_ids=list(range(N_CORES)),
                               trace=TRACE)
    global LAST_EXEC_NS, LAST_PROFILE
    LAST_EXEC_NS = res.exec_time_ns
    LAST_PROFILE = res.profile_json

    outs = []
    for c in range(N_CORES):
        o = res.results[c]["out"]
        den = res.results[c]["den"].reshape(bags_core, 1)
        outs.append(o / den)
    return np.concatenate(outs, axis=0).astype(np.float32)


# revision 18
# speedup vs baseline: 1.0061x; 1.0061x over previous
"""AttentionPooling Bass kernel for 8 TRN2 NeuronCores.

Problem: x [262144, 1024] f32, bags of 128 consecutive rows (2048 bags).
  scores = (tanh(x @ W1 + b1) @ W2 + b2)[:, 0]        per-row MLP score
  w      = softmax(scores) within each bag
  out[b] = sum_i w[i] * x[i]  over the bag's rows  -> [2048, 1024] f32

Sharding: data-parallel over bags; core c gets bags [c*256, (c+1)*256).
Weights replicated. No cross-core communication. b2 is dropped (uniform
shift inside each bag's softmax — a no-op for the output).

Numerics: |score| <= ||W2||_1 < 26 (tanh-bounded), so exp() cannot
overflow f32 and the softmax max-subtraction is skipped entirely.
Normalization is deferred to the HOST: the device returns per-bag
unnormalized weighted sums plus the per-bag sum of exp (den); kernel()
divides on the way out.

Mixed precision phase 1 (the ~89%-of-PE-time term): the last M_FP8
bags of each 8-bag group run their score matmul fully in fp8-e4m3
with DoubleRow (K=256 per matmul at 2 elem/cycle — 2x bf16), the rest
in bf16. Bag-granular (not chunk-granular) keeps the PE dtype
transitions to ~2 per group — interleaving dtypes per-bag measurably
costs ~200ns per transition. Operands are pre-scaled (xt*16, W1*512 —
exact pow2 shifts) so both dtypes produce PSUM = 8192*h and tanh
applies scale=1/8192. fp8 on a fraction f of the bags costs output
rel err ~ 0.026*sqrt(f) (numpy-sim, HW-confirmed) vs the 2e-2 gate:
M_FP8=4 measures 0.0181.

Pipeline notes (learned from traces):
- out-row/den DMAs live on the gpsimd queue: on the sync queue they
  head-of-line blocked the next group's input DMAs at every group
  boundary (the sync queue runs ~10us ahead, in order).
- gpsimd.partition_broadcast loads a GPSIMD ucode library whose
  LOAD_LIB + 17us DRAIN stalls the gpsimd DMA queue at startup; w2 is
  replicated with a stride-0 broadcast DMA read instead.
- the softmax chain is split per QUARTET (4 bags): exp of bags
  [4q,4q+4) only needs those bags' score reductions, so quartet 0 of
  group g runs at bag 6 of group g and quartet 1 at bag 2 of group
  g+1 — no cross-engine chain dangles off the group boundary.
"""

import sys

if "/opt/trn_rl_repo" not in sys.path:
    sys.path.insert(0, "/opt/trn_rl_repo")

import numpy as np

import concourse.bass as bass
import concourse.bacc as bacc
import concourse.mybir as mybir
import concourse.tile as tile
from concourse.bass_utils import run_bass_kernel_spmd

F32 = mybir.dt.float32
BF16 = mybir.dt.bfloat16
FP8 = mybir.dt.float8e4
AF = mybir.ActivationFunctionType
ALU = mybir.AluOpType
DR = mybir.MatmulPerfMode.DoubleRow

N_CORES = 8
BAG = 128
D = 1024
H = 1024
DC = D // 128   # contraction chunks
GROUP = 8       # bags per group
WG = 4          # bags per weighted-sum quartet (PSUM col-group packing)

# bags per group (of 8) whose score matmul runs fully in fp8-e4m3
# DoubleRow (the LAST M_FP8 bags of each group — consecutive to
# minimize PE dtype switches, last so the first bags only need the
# bf16 weights that stream first). 0 = pure bf16.
# Output rel err ~ sqrt(M_FP8/8)*0.026.
M_FP8 = 4

SX = 16.0       # x pre-scale (exact pow2)
SW = 512.0      # W1 pre-scale (exact pow2)

# set by test.py for profiling; the grading harness leaves these alone
TRACE = False
LAST_EXEC_NS = None
LAST_PROFILE = None

_cache = {}


def _build(bags_core: int, with_b1: bool, m_fp8: int, n_cores: int = N_CORES):
    """Build the per-core Bass module. All cores run the same NEFF."""
    assert bags_core % GROUP == 0 and GROUP % WG == 0
    assert 0 <= m_fp8 <= GROUP
    rows_core = bags_core * BAG
    n_groups = bags_core // GROUP
    use_fp8 = m_fp8 > 0
    use_bf = m_fp8 < GROUP

    nc = bacc.Bacc("TRN2", target_bir_lowering=False, debug=False,
                   num_devices=n_cores)
    x_h = nc.declare_dram_parameter("x", [rows_core, D], BF16, isOutput=False)
    if use_bf:
        xtb_h = nc.declare_dram_parameter("xtb", [rows_core, D], BF16,
                                          isOutput=False)
        w1b_h = nc.declare_dram_parameter("w1b", [D, H], BF16, isOutput=False)
    if use_fp8:
        xtf_h = nc.declare_dram_parameter("xtf", [rows_core, D], FP8,
                                          isOutput=False)
        w1f_h = nc.declare_dram_parameter("w1f", [D, H], FP8, isOutput=False)
    w2_h = nc.declare_dram_parameter("w2", [1, H], BF16, isOutput=False)
    if with_b1:
        b1_h = nc.declare_dram_parameter("b1", [1, H], BF16, isOutput=False)
    out_h = nc.declare_dram_parameter("out", [bags_core, D], F32, isOutput=True)
    den_h = nc.declare_dram_parameter("den", [bags_core, 1], F32, isOutput=True)

    with tile.TileContext(nc) as tc:
        with (
            tc.tile_pool(name="const", bufs=1) as const_pool,
            tc.tile_pool(name="xtb", bufs=6) as xtb_pool,
            tc.tile_pool(name="xtf", bufs=6) as xtf_pool,
            tc.tile_pool(name="xb", bufs=16) as xb_pool,
            tc.tile_pool(name="tanh", bufs=2) as t_pool,
            tc.tile_pool(name="dump", bufs=1) as dump_pool,
            tc.tile_pool(name="scores", bufs=2) as sc_pool,
            tc.tile_pool(name="e4", bufs=3) as e_pool,
            tc.tile_pool(name="den", bufs=3) as den_pool,
            tc.tile_pool(name="ystage", bufs=2) as y_pool,
            tc.tile_pool(name="ps_s", bufs=2, space="PSUM") as ps_s_pool,
            tc.tile_pool(name="ps_y", bufs=3, space="PSUM") as ps_y_pool,
            tc.tile_pool(name="ps_d", bufs=1, space="PSUM") as ps_d_pool,
        ):
            # ---- constants / weights (resident) ----
            ones_col = const_pool.tile([128, 1], BF16)
            nc.any.memset(ones_col[:, :], 1.0)

            # Weight DMA order matches first use: bag 0 (bf16) sweeps
            # w1b chunks 0..7 of the j=0 half first, so stream those
            # halves before anything else; w1f is first needed by bag
            # 8-M_FP8 (~4 bag-periods in).
            if use_bf:
                w1b_sb = const_pool.tile([128, DC, H], BF16)
                for j in range(2):
                    for c in range(DC):
                        nc.gpsimd.dma_start(
                            out=w1b_sb[:, c, 512 * j:512 * (j + 1)],
                            in_=w1b_h[c * 128:(c + 1) * 128,
                                      512 * j:512 * (j + 1)])
            if use_fp8:
                w1f_sb = const_pool.tile([128, DC, H], FP8)
                for c in range(DC):
                    nc.gpsimd.dma_start(out=w1f_sb[:, c, :],
                                        in_=w1f_h[c * 128:(c + 1) * 128, :])

            # replicate W2 across partitions with a stride-0 DMA read
            w2_rep = const_pool.tile([128, H], BF16)
            nc.gpsimd.dma_start(out=w2_rep[:, :],
                                in_=w2_h[:, :].broadcast_to([128, H]))

            if with_b1:
                b1_row = const_pool.tile([1, H], BF16)
                nc.gpsimd.dma_start(out=b1_row[:, :], in_=b1_h[:, :])
                ones_row = const_pool.tile([1, 128], BF16)
                nc.any.memset(ones_row[:, :], 1.0)

            def phase1(g, sc_tile, xbs, mid_cbs):
                """Score matmuls+tanh+projection for group g's 8 bags.

                Fills sc_tile [128, 8] (per-row scores, column=bag) and
                appends the bags' x tiles to xbs. mid_cbs: {bag_idx:
                [callbacks]} invoked between bags to stage quartet
                work inside this group's matmul stream.
                """
                for n in range(GROUP):
                    for cb in mid_cbs.get(n, ()):
                        cb()
                    bag = g * GROUP + n
                    is8 = n >= GROUP - m_fp8
                    rs = slice(bag * BAG, (bag + 1) * BAG)
                    if is8:
                        xt_t = xtf_pool.tile([128, DC, 128], FP8)
                        nc.sync.dma_start(out=xt_t[:, :, :], in_=xtf_h[rs, :])
                    else:
                        xt_t = xtb_pool.tile([128, DC, 128], BF16)
                        nc.sync.dma_start(out=xt_t[:, :, :], in_=xtb_h[rs, :])
                    x_b = xb_pool.tile([128, D], BF16)
                    nc.sync.dma_start(out=x_b[:, :], in_=x_h[rs, :])
                    xbs.append(x_b)

                    ps_s = ps_s_pool.tile([128, 2, 512], F32)
                    for j in range(2):
                        ps_j = ps_s[:, j, :]
                        hs = slice(512 * j, 512 * (j + 1))
                        if is8:
                            for p in range(DC // 2):
                                nc.tensor.matmul(ps_j[:, :],
                                                 lhsT=xt_t[:, 2 * p:2 * p + 2, :],
                                                 rhs=w1f_sb[:, 2 * p:2 * p + 2, hs],
                                                 start=(p == 0),
                                                 stop=(p == DC // 2 - 1
                                                       and not with_b1),
                                                 perf_mode=DR)
                        else:
                            for c in range(DC):
                                nc.tensor.matmul(ps_j[:, :],
                                                 lhsT=xt_t[:, c, :],
                                                 rhs=w1b_sb[:, c, hs],
                                                 start=(c == 0),
                                                 stop=(c == DC - 1
                                                       and not with_b1))
                        if with_b1:
                            nc.tensor.matmul(ps_j[:, :], lhsT=ones_row[:, :],
                                             rhs=b1_row[:, hs],
                                             start=False, stop=True)
                    t_t = t_pool.tile([128, H], BF16)
                    # one activation spanning both PSUM banks: ScalarE is
                    # near-saturated during the fp8 half, and per-op
                    # overhead (~150ns) on 2 ops/bag was gating the ps_s
                    # recycle that the next bags' matmuls wait on
                    nc.scalar.activation(t_t[:, :], ps_s[:, :, :], AF.Tanh,
                                         scale=1.0 / (SX * SW))

                    dump = dump_pool.tile([128, H], BF16)
                    nc.vector.tensor_mul(dump[:, :], t_t[:, :], w2_rep[:, :])
                    nc.vector.reduce_sum(sc_tile[:, n:n + 1], dump[:, :],
                                         axis=mybir.AxisListType.X)

            def q_prep(q, sc_tile, e8):
                """exp of bags [4q,4q+4) into e8[:, q, :], plus the 8x
                column-replicated copy the weighted-sum matmuls load:
                col c of the M=32 stationary holds e[:, c mod 4]; bag
                (q,v)'s out row lands at partition 32v + (4q+v) as
                (4q+v) mod 4 == v."""
                nc.scalar.activation(e8[:, q, :],
                                     sc_tile[:, q * WG:(q + 1) * WG], AF.Exp)
                e32_t = e_pool.tile([128, GROUP, WG], BF16)
                nc.vector.tensor_copy(
                    e32_t[:, :, :],
                    e8[:, q, :].unsqueeze(1).broadcast_to([128, GROUP, WG]))
                return e32_t

            def q_wsum(g, q, xbs, e32_t, e8, last=False):
                """weighted sums for bags [4q,4q+4) of group g; quartet 1
                also emits the group's denominator matmul."""
                e32 = e32_t[:, :, :]
                if q == 1:
                    ps_d = ps_d_pool.tile([128, 1], F32)
                    nc.tensor.matmul(ps_d[0:GROUP, :], lhsT=e8[:, :, :],
                                     rhs=ones_col[:, :], start=True, stop=True)
                    dstage = den_pool.tile([GROUP, 1], F32)
                    nc.vector.tensor_copy(dstage[:, :], ps_d[0:GROUP, :])
                    b0 = g * GROUP
                    nc.gpsimd.dma_start(out=den_h[b0:b0 + GROUP, :],
                                        in_=dstage[:, :])

                ys = y_pool.tile([128, D], F32)
                for j in range(2):
                    ps_y = ps_y_pool.tile([128, 512], F32)
                    for v in range(WG):
                        b = q * WG + v
                        nc.tensor.matmul(ps_y[32 * v:32 * v + 32, :],
                                         lhsT=e32,
                                         rhs=xbs[b][:, 512 * j:512 * (j + 1)],
                                         start=True, stop=True,
                                         tile_position=(0, 32 * v))
                    # Alternate ScalarE/VectorE so the two banks drain in
                    # parallel (bank fully written so ScalarE is safe).
                    if j == 0:
                        nc.vector.tensor_copy(ys[:, 0:512], ps_y[:, :])
                    else:
                        nc.scalar.copy(ys[:, 512:1024], ps_y[:, :])
                # spread the final quartet's row DMAs across queues so the
                # kernel tail isn't serialized on one DMA queue
                engines = ([nc.gpsimd, nc.sync, nc.scalar, nc.gpsimd]
                           if last else [nc.gpsimd] * WG)
                for v in range(WG):
                    bag = g * GROUP + q * WG + v
                    p = 32 * v + q * WG + v
                    engines[v].dma_start(out=out_h[bag:bag + 1, :],
                                         in_=ys[p:p + 1, :])

            # Pipeline: quartet 0 of group g: exp+replicate at bag 5
            # (bags 0-3's reductions are just done), weighted-sum
            # matmuls at bag 0 of group g+1 — their bf16 instructions
            # sit exactly at the fp8->bf16 group boundary, adding NO
            # extra PE dtype transitions (placing them at bag 7, mid
            # fp8 run, cost ~2 transitions x ~190ns per group).
            # Quartet 1 of group g: exp+replicate at bag 1 of g+1,
            # weighted sums + denominator at bag 3 of g+1.
            prev = None
            for g in range(n_groups):
                sc_tile = sc_pool.tile([128, GROUP], F32)
                e8 = e_pool.tile([128, 2, WG], BF16, name="e8")
                xbs = []
                cell = {}

                def p0(s=sc_tile, e=e8, c=cell):
                    c["e0"] = q_prep(0, s, e)

                cbs = {5: [p0]}
                if prev is not None:
                    pg, psc, pxbs, pe8, pcell = prev

                    def w0(pg=pg, x=pxbs, e=pe8, c=pcell):
                        q_wsum(pg, 0, x, c["e0"], e)

                    def p1(s=psc, e=pe8, c=pcell):
                        c["e1"] = q_prep(1, s, e)

                    def w1(pg=pg, x=pxbs, e=pe8, c=pcell):
                        q_wsum(pg, 1, x, c["e1"], e)

                    cbs[0] = [w0]
                    cbs[1] = [p1]
                    cbs[3] = [w1]
                phase1(g, sc_tile, xbs, cbs)
                prev = (g, sc_tile, xbs, e8, cell)
            pg, psc, pxbs, pe8, pcell = prev
            q_wsum(pg, 0, pxbs, pcell["e0"], pe8)
            e1 = q_prep(1, psc, pe8)
            q_wsum(pg, 1, pxbs, e1, pe8, last=True)

    nc.finalize()
    return nc


def _numpy_fallback(x, W1, b1, W2, b2, bag_sizes):
    seg_ends = np.cumsum(bag_sizes)
    seg_starts = seg_ends - bag_sizes
    scores = (np.tanh(x @ W1 + b1) @ W2 + b2)[:, 0]
    out = np.zeros((bag_sizes.shape[0], x.shape[1]), dtype=x.dtype)
    for i, (s, e) in enumerate(zip(seg_starts, seg_ends)):
        sc = scores[s:e]
        w = np.exp(sc - sc.max())
        w /= w.sum()
        out[i] = w @ x[s:e]
    return out


def _host_prep(x, n_bags, m_fp8):
    """bf16 cast of x, plus the per-bag-transposed scaled copies
    (bf16 and/or fp8 depending on the bag mix) via jax CPU."""
    import jax
    import jax.numpy as jnp
    import ml_dtypes

    cpu = jax.devices("cpu")[0]
    with jax.default_device(cpu):
        xj = jnp.asarray(x)
        xb = np.asarray(xj.astype(jnp.bfloat16))
        xt = ((xj * SX).reshape(n_bags, BAG, DC, 128).transpose(0, 3, 2, 1)
              .reshape(n_bags * BAG, D))
        xtb = xtf = None
        if m_fp8 < GROUP:
            xtb = np.asarray(xt.astype(jnp.bfloat16))
        if m_fp8 > 0:
            xtf = np.asarray(xt).astype(ml_dtypes.float8_e4m3)
        return xb, xtb, xtf


def kernel(x, W1, b1, W2, b2, bag_sizes):
    x = np.ascontiguousarray(np.asarray(x, dtype=np.float32))
    W1 = np.asarray(W1, dtype=np.float32)
    b1 = np.asarray(b1, dtype=np.float32)
    W2 = np.asarray(W2, dtype=np.float32)
    b2 = np.asarray(b2, dtype=np.float32)
    bag_sizes = np.asarray(bag_sizes)

    n_bags = bag_sizes.shape[0]
    if not (np.all(bag_sizes == BAG) and x.shape[0] == n_bags * BAG
            and x.shape[1] == D and n_bags % (N_CORES * GROUP) == 0):
        return _numpy_fallback(x, W1, b1, W2, b2, bag_sizes)

    bags_core = n_bags // N_CORES
    rows_core = bags_core * BAG
    with_b1 = bool(np.any(b1))

    key = (bags_core, with_b1, M_FP8)
    if key not in _cache:
        _cache[key] = _build(bags_core, with_b1, M_FP8)
    nc = _cache[key]

    import ml_dtypes
    x_bf, xtb, xtf = _host_prep(x, n_bags, M_FP8)
    w1s = W1 * SW
    w2_row = np.ascontiguousarray(W2.reshape(1, H)).astype(ml_dtypes.bfloat16)
    in_maps = []
    for c in range(N_CORES):
        rs = slice(c * rows_core, (c + 1) * rows_core)
        im = {"x": x_bf[rs], "w2": w2_row}
        if M_FP8 < GROUP:
            im["xtb"] = xtb[rs]
            im["w1b"] = w1s.astype(ml_dtypes.bfloat16)
        if M_FP8 > 0:
            im["xtf"] = xtf[rs]
            im["w1f"] = w1s.astype(ml_dtypes.float8_e4m3)
        if with_b1:
            im["b1"] = np.ascontiguousarray(
                (b1 * SX * SW).reshape(1, H)).astype(ml_dtypes.bfloat16)
        in_maps.append(im)

    res = run_bass_kernel_spmd(nc, in_maps, core_ids=list(range(N_CORES)),
                               trace=TRACE)
    global LAST_EXEC_NS, LAST_PROFILE
    LAST_EXEC_NS = res.exec_time_ns
    LAST_PROFILE = res.profile_json

    outs = []
    for c in range(N_CORES):
        o = res.results[c]["out"]
        den = res.results[c]["den"].reshape(bags_core, 1)
        outs.append(o / den)
    return np.concatenate(outs, axis=0).astype(np.float32)
